# revision 5
# baseline (speedup 1.0000x reference)
"""CoRNN kernel for 8x Trainium2 NeuronCores (Bass/Tile).

Data-parallel over batch (B=64 -> 8 per core). Per core:
  phase A: px[b,t,h] = x[b,t,:] @ Wx.T + bias   (f32r matmuls, stored bf16)
  phase B: 1024 sequential steps of the CoRNN cell with scaled state
           Z = hz/dt, Y = hy:
             pre = px_t + Z @ (dt*Wz).T + Y @ Wy.T
             Z' = (1-dt*eps)*Z + (-gamma)*Y + tanh(pre)
             Y' = Y + dt^2 * Z'
  The recurrent matmuls are state-stationary (lhsT = state tile [128,8]),
  weight-moving (rhs = [128,128] bf16), 4-way col-tiled via tile_position
  so 4 matmuls run concurrently on the PE array.

h-index encoding: h = 128*j + 32*g + cc  (g = col-group, j,cc in [0,4)x[0,32))
"""

import sys
import numpy as np
import ml_dtypes

sys.path.insert(0, "/opt/trn_rl_repo")

import concourse.bass as bass  # noqa: E402
import concourse.tile as tile  # noqa: E402
from concourse import bacc, mybir  # noqa: E402
from concourse.bass_utils import run_bass_kernel_spmd  # noqa: E402
from contextlib import ExitStack  # noqa: E402

F32 = mybir.dt.float32
F32R = mybir.dt.float32r
BF16 = mybir.dt.bfloat16
AF = mybir.ActivationFunctionType
ALU = mybir.AluOpType

DT, GAMMA, EPS = 0.054, 4.9, 4.8
B, I, H = 64, 256, 512
NCORES = 8
BC = B // NCORES  # 8 batch rows per core
A_ = 1.0 - DT * EPS
DT2 = DT * DT

_CACHE = {}
_last_in_maps = None


def build_nc(T=1024, Tc=64, repeat=1):
    """Build the per-core Bass program. repeat>1 re-runs phase B (timing)."""
    assert T % Tc == 0 and Tc % 4 == 0
    nc = bacc.Bacc("TRN2", target_bir_lowering=False, debug=False,
                   num_devices=NCORES)

    xt_d = nc.dram_tensor("xt", [2 * 128, BC * T], F32R, kind="ExternalInput").ap()
    wa_d = nc.dram_tensor("wa", [128, 2 * 4 * 128], F32R, kind="ExternalInput").ap()
    wb_d = nc.dram_tensor("wb", [128, 8 * 4 * 128], BF16, kind="ExternalInput").ap()
    bias_d = nc.dram_tensor("biasv", [128, 4], F32, kind="ExternalInput").ap()
    idb_d = nc.dram_tensor("idb", [128, 128], BF16, kind="ExternalInput").ap()
    idf_d = nc.dram_tensor("idf", [128, 128], F32, kind="ExternalInput").ap()
    outs_d = nc.dram_tensor("outs", [T // 4, 128, 128], F32, kind="ExternalOutput").ap()

    with tile.TileContext(nc) as tc, ExitStack() as ctx:
        cpool = ctx.enter_context(tc.tile_pool(name="consts", bufs=1))
        papool = ctx.enter_context(tc.tile_pool(name="pa", bufs=4, space="PSUM"))
        pbpool = ctx.enter_context(tc.tile_pool(name="pb", bufs=2, space="PSUM"))
        ptpool = ctx.enter_context(tc.tile_pool(name="pt", bufs=2, space="PSUM"))
        wpool = ctx.enter_context(tc.tile_pool(name="work", bufs=3))

        # ---- constants / big buffers
        xt = cpool.tile([128, 2 * BC * T], F32R)        # x.T   [i%128, (ihi, b, t)]
        wa = cpool.tile([128, 2 * 4 * 128], F32R)        # phase A stationary
        wb = cpool.tile([128, 8 * 4 * 128], BF16)        # phase B moving weights
        biasv = cpool.tile([128, 4], F32)
        idb = cpool.tile([128, 128], BF16)               # identity bf16 (px inject)
        idf = cpool.tile([128, 128], F32)                # identity f32 (hist transpose)
        pxT = cpool.tile([128, 4 * BC * T], BF16)        # px  [m=(j,cc), (g, b, t)]
        zb = [cpool.tile([128, 128], BF16, name=f"zbuf{i}") for i in range(2)]
        yb = [cpool.tile([128, 128], BF16, name=f"ybuf{i}") for i in range(2)]
        hist = [cpool.tile([128, Tc * 32], F32, name=f"histbuf{i}") for i in range(2)]
        yzero = cpool.tile([128, 32], F32)
        u_t = cpool.tile([128, 32], F32)

        xt_v = xt[:].rearrange("p (ihi c) -> p ihi c", ihi=2)
        for ihi in range(2):
            nc.sync.dma_start(xt_v[:, ihi, :], xt_d[ihi * 128:(ihi + 1) * 128, :])
        nc.sync.dma_start(wa[:], wa_d[:])
        nc.sync.dma_start(wb[:], wb_d[:])
        nc.sync.dma_start(biasv[:], bias_d[:])
        nc.sync.dma_start(idb[:], idb_d[:])
        nc.sync.dma_start(idf[:], idf_d[:])

        nc.gpsimd.memset(zb[1][:], 0.0)
        nc.gpsimd.memset(yb[1][:], 0.0)
        nc.gpsimd.memset(yzero[:], 0.0)

        wa_v = wa[:].rearrange("p (it g m) -> p it g m", it=2, g=4)
        wb_v = wb[:].rearrange("p (kt g n) -> p kt g n", kt=8, g=4)
        pxT_v = pxT[:].rearrange("p (g b t) -> p g b t", g=4, b=BC)

        # ---- phase A: px = x @ Wx.T + bias  -> pxT (bf16)
        ntc = T // 512 if T >= 512 else 1
        tw = min(512, T)
        k = 0
        for g in range(4):
            for bb in range(BC):
                for tcn in range(ntc):
                    pa = papool.tile([128, tw], F32, tag="pa")
                    for it in range(2):
                        nc.tensor.matmul(
                            pa[:],
                            wa_v[:, it, g, :],
                            xt_v[:, it, bb * T + tcn * tw: bb * T + (tcn + 1) * tw],
                            start=(it == 0), stop=(it == 1))
                    dst = pxT_v[:, g, bb, tcn * tw:(tcn + 1) * tw]
                    if k % 2 == 0:
                        nc.scalar.activation(dst, pa[:], AF.Identity,
                                             bias=biasv[:, g:g + 1])
                    else:
                        nc.vector.tensor_scalar(dst, pa[:], biasv[:, g:g + 1],
                                                None, op0=ALU.add)
                    k += 1

        # ---- phase B
        for rep in range(repeat):
            for t in range(T):
                par = t % 2
                zprev, ynow = zb[1 - par], zb[par]
                pre = pbpool.tile([128, 128], F32, tag="pre")
                # 36 matmuls: waves over (px, 8 state k-tiles) x 4 col groups
                for g in range(4):
                    nc.tensor.matmul(
                        pre[32 * g:32 * g + BC, :],
                        pxT_v[:, g, :, t], idb[:],
                        start=True, stop=False, tile_position=(0, 32 * g))
                for kt in range(8):
                    st = zb[1 - par] if kt < 4 else yb[1 - par]
                    c0 = 32 * (kt % 4)
                    for g in range(4):
                        nc.tensor.matmul(
                            pre[32 * g:32 * g + BC, :],
                            st[:, c0:c0 + BC], wb_v[:, kt, g, :],
                            start=False, stop=(kt == 7), tile_position=(0, 32 * g))

                th = wpool.tile([128, 128], F32, tag="th")
                nc.scalar.activation(th[:], pre[:], AF.Tanh)
                tht = wpool.tile([128, 128], F32, tag="tht")
                nc.vector.transpose(tht[:], th[:])

                hb = (t // Tc) % 2
                sl = t % Tc
                if t == 0:
                    ypv = yzero[:].rearrange("p (j b) -> p j b", j=4)
                else:
                    phb = ((t - 1) // Tc) % 2
                    psl = (t - 1) % Tc
                    ypv = hist[phb][:, psl * 32:(psl + 1) * 32].rearrange(
                        "p (j b) -> p j b", j=4)
                tht_vv = tht[:].rearrange("p (j q) -> p j q", j=4)[:, :, 0:BC]
                uv = u_t[:].rearrange("p (j b) -> p j b", j=4)
                nc.vector.scalar_tensor_tensor(
                    uv, ypv, -GAMMA, tht_vv, op0=ALU.mult, op1=ALU.add)
                zpv = zprev[:].rearrange("p (kt q) -> p kt q", kt=4)[:, :, 0:BC]
                znv = ynow[:].rearrange("p (kt q) -> p kt q", kt=4)[:, :, 0:BC]
                nc.vector.scalar_tensor_tensor(
                    znv, zpv, A_, uv, op0=ALU.mult, op1=ALU.add)
                hsl = hist[hb][:, sl * 32:(sl + 1) * 32].rearrange(
                    "p (j b) -> p j b", j=4)
                nc.vector.scalar_tensor_tensor(
                    hsl, znv, DT2, ypv, op0=ALU.mult, op1=ALU.add)
                ybv = yb[par][:].rearrange("p (kt q) -> p kt q", kt=4)[:, :, 0:BC]
                nc.scalar.copy(ybv, hsl)

                if t % 4 == 3 and rep == repeat - 1:
                    bi = t // 4
                    blk = hist[hb][:, (sl - 3) * 32:(sl + 1) * 32]
                    ptr = ptpool.tile([128, 128], F32, tag="ptr")
                    nc.tensor.transpose(ptr[:], blk, idf[:])
                    stg = wpool.tile([128, 128], F32, tag="stg")
                    if (t // 4) % 2 == 0:
                        nc.scalar.copy(stg[:], ptr[:])
                    else:
                        nc.vector.tensor_copy(stg[:], ptr[:])
                    nc.sync.dma_start(outs_d[bi], stg[:])

    nc.compile()
    return nc


def _prep_consts(W, b):
    Wx, Wz, Wy = W[:, :I], W[:, I:I + H], W[:, I + H:]
    Wcat = np.concatenate([DT * Wz, Wy], axis=1)  # [H, 2H] (scaled Z part)

    m = np.arange(128)
    jj, cc = m // 32, m % 32

    wa = np.zeros((128, 2, 4, 128), np.float32)
    for it in range(2):
        for g in range(4):
            h = 128 * jj + 32 * g + cc          # [128] h for each m
            wa[:, it, g, :] = Wx[h, it * 128:(it + 1) * 128].T
    wa = wa.reshape(128, -1)

    wb = np.zeros((128, 8, 4, 128), np.float32)
    p = np.arange(128)
    for kt in range(8):
        kk = (kt % 4) * 128 + p                  # contraction h-index
        for g in range(4):
            h = 128 * jj + 32 * g + cc
            src = Wcat[:, :H] if kt < 4 else Wcat[:, H:]
            wb[:, kt, g, :] = src[h][:, kk].T    # [128(p), 128(n)]
    wb = wb.reshape(128, -1).astype(ml_dtypes.bfloat16)

    biasv = np.zeros((128, 4), np.float32)
    for g in range(4):
        biasv[:, g] = b[128 * jj + 32 * g + cc]

    idb = np.eye(128, dtype=ml_dtypes.bfloat16)
    idf = np.eye(128, dtype=np.float32)
    return wa, wb, biasv, idb, idf


def kernel(x, W, b):
    x = np.asarray(x, np.float32)
    W = np.asarray(W, np.float32)
    b = np.asarray(b, np.float32)
    T = x.shape[1]

    key = T
    if key not in _CACHE:
        _CACHE[key] = build_nc(T=T)
    nc = _CACHE[key]

    wa, wb, biasv, idb, idf = _prep_consts(W, b)
    in_maps = []
    for c in range(NCORES):
        xc = x[c * BC:(c + 1) * BC]                      # [BC, T, I]
        xt = np.ascontiguousarray(xc.transpose(2, 0, 1).reshape(I, BC * T))
        in_maps.append({"xt": xt, "wa": wa, "wb": wb, "biasv": biasv,
                        "idb": idb, "idf": idf})

    global _last_in_maps
    _last_in_maps = in_maps
    res = run_bass_kernel_spmd(nc, in_maps, list(range(NCORES)))
    parts = []
    for c in range(NCORES):
        od = res.results[c]["outs"].reshape(T // 4, 4, 4, BC, 128)
        # [tb, t4, j, b, p] -> [b, t, h=128j+p]
        parts.append(np.ascontiguousarray(od.transpose(3, 0, 1, 2, 4)).reshape(BC, T, H))
    outs = np.concatenate(parts, axis=0)
    h_out = np.ascontiguousarray(outs[:, -1, :][None])
    return outs, h_out


if __name__ == "__main__":
    rng = np.random.default_rng(0)
    T = int(sys.argv[1]) if len(sys.argv) > 1 else 64
    x = rng.standard_normal((B, T, I), dtype=np.float32)
    W = (rng.standard_normal((H, I + 2 * H)) / np.sqrt(I + 2 * H)).astype(np.float32)
    b = (rng.standard_normal(H) * 0.01).astype(np.float32)

    outs, h_out = kernel(x, W, b)

    # numpy reference
    Wx, Wz, Wy = W[:, :I], W[:, I:I + H], W[:, I + H:]
    px = np.einsum('bti,hi->tbh', x, Wx, dtype=np.float32) + b
    hy = np.zeros((B, H), np.float32)
    hz = np.zeros((B, H), np.float32)
    routs = np.zeros((T, B, H), np.float32)
    for t in range(T):
        pre = px[t] + hz @ Wz.T + hy @ Wy.T
        hz = hz + DT * (np.tanh(pre) - GAMMA * hy - EPS * hz)
        hy = hy + DT * hz
        routs[t] = hy
    routs = routs.transpose(1, 0, 2)
    err = np.linalg.norm((outs - routs).ravel()) / max(np.linalg.norm(routs.ravel()), 1e-30)
    print("l2 rel err:", err)
    print("absmax:", np.abs(outs - routs).max(), " ref absmax:", np.abs(routs).max())
